# revision 37
# baseline (speedup 1.0000x reference)
"""Causal depthwise-conv MLP block (W_in -> causal conv K=4 -> SiLU -> W_out)
as a Bass/Tile kernel running data-parallel on 8 Trainium2 NeuronCores.

Sharding: (batch=4) x (sequence halves=2) -> 8 shards of 2048 sequence rows.
The causal conv needs 3 columns of left context; those are computed on the
host (exact fp32) and passed per-core, with the input-projection bias b_in
folded out of the conv input and into the SiLU bias so zero-padding at batch
starts is exact.

The input projection runs as level-1 Strassen per 1024-column super-chunk:
W_in is 2x2-blocked (channel halves x hidden halves), the moving operand is
2x2-blocked (hidden halves x the two 512-col halves of the super), and the 7
Strassen products are recombined into the four output quadrants while they
sit in PSUM. This removes 1/8 of the tensor-engine cycles of the projection.
The 5 non-trivial moving combos are built once per super on the DVE in bf16;
the 7 stationary combos are precomputed on the host. Product order
[M2,M5,M1,M4,M7,M3,M6] tracks the hst DMA batch arrival and lets each PSUM
bank be consumed eagerly:
  X11 = -M5+M1+M4+M7, X12 = M5+M3, X21 = M2+M4, X22 = M1-M2+M3+M6.

On-chip layout is channel-major ([C,seq] on partitions) so the depthwise conv
is per-partition multiply-accumulate along the free dim (4 fused
scalar_tensor_tensor ops per channel tile).
"""

import os

# Recover cleanly if a previous (crashed) run left the NeuronCores wedged
# (NRT_EXEC_UNIT_UNRECOVERABLE). Must be set before the runtime initializes.
os.environ.setdefault("NEURON_RT_RESET_CORES", "1")

import numpy as np
import ml_dtypes

P = 128
B, S, H, C, K = 4, 4096, 2048, 4096, 4
NCORES = 8
N = S // 2          # sequence rows per core
KH = H // P         # 16 contraction tiles for the input projection
CT = C // P         # 32 channel tiles
MT = H // P         # 16 output row tiles
SUP = 1024          # sequence super-chunk held in SBUF as Y
NSUP = N // SUP     # 2
SUB = 512           # matmul moving free dim / PSUM bank
NSUB = SUP // SUB   # 2
RT = 16             # row tiles per Strassen quadrant (2048/128)
KQ = 8              # contraction tiles per Strassen K-half (1024/128)

# Strassen product schedule, ordered so each product's moving operand is
# ready as the hst DMA batches land: M2 needs batch1 (hA 0-7), M5 batch2
# (hB 8-15), M1's combo T1 batches 1+2, M4/M7 batch3 (hA 8-15), M3/M6
# batch4 (hB 0-7).
PRODS = ["M2", "M5", "M1", "M4", "M7", "M3", "M6"]
ST_RING_SPLIT = False
PRE_ACT_EARLY = (0, 1, 2, 3, 4)  # prologue stationaries on Act ring before b2
PRE_ACT_LATE = ()                # ... and after b2

_NC = None
LAST_RESULT = None


DEFAULT_BUFS = dict(hs=1, sp=6, wo=2, xs=3, ya=3, ob=2, psA=6, psB=2)


def _build_nc(bufs=None):
    import concourse.bacc as bacc
    import concourse.mybir as mybir
    from concourse.tile import TileContext
    from contextlib import ExitStack

    nb = dict(DEFAULT_BUFS)
    if bufs:
        nb.update(bufs)

    fp32 = mybir.dt.float32
    bf16 = mybir.dt.bfloat16
    AF = mybir.ActivationFunctionType
    ALU = mybir.AluOpType

    nc = bacc.Bacc()
    hsT = nc.declare_dram_parameter("hsT", [H, N], bf16, isOutput=False)
    w1s = nc.declare_dram_parameter("w1s", [7 * RT, P, KQ * P], bf16, isOutput=False)
    w_out = nc.declare_dram_parameter("w_out", [MT, P, CT * P], bf16, isOutput=False)
    convw = nc.declare_dram_parameter("convw", [P, CT * 4], fp32, isOutput=False)
    biasf = nc.declare_dram_parameter("biasf", [P, CT], fp32, isOutput=False)
    halo = nc.declare_dram_parameter("halo", [P, CT * 3], fp32, isOutput=False)
    bout = nc.declare_dram_parameter("bout", [P, MT], fp32, isOutput=False)
    outT = nc.declare_dram_parameter("outT", [H, N], fp32, isOutput=True)

    with TileContext(nc) as tc, ExitStack() as ctx:
        const = ctx.enter_context(tc.tile_pool(name="const", bufs=1))
        hs_pool = ctx.enter_context(tc.tile_pool(name="hs", bufs=nb["hs"]))
        tc_pool = ctx.enter_context(tc.tile_pool(name="tcomb", bufs=1))
        sp_pool = ctx.enter_context(tc.tile_pool(name="sp", bufs=nb["sp"]))
        wo_pool = ctx.enter_context(tc.tile_pool(name="wo", bufs=nb["wo"]))
        xs_pool = ctx.enter_context(tc.tile_pool(name="xs", bufs=nb["xs"]))
        ya_pool = ctx.enter_context(tc.tile_pool(name="ya", bufs=nb["ya"]))
        yb_pool = ctx.enter_context(tc.tile_pool(name="yb", bufs=1))
        ob_pool = ctx.enter_context(tc.tile_pool(name="ob", bufs=nb["ob"]))
        psA = ctx.enter_context(tc.tile_pool(name="psA", bufs=nb["psA"], space="PSUM"))
        psB = ctx.enter_context(tc.tile_pool(name="psB", bufs=nb["psB"], space="PSUM"))

        cw = const.tile([P, CT * 4], fp32, tag="cw")
        nc.sync.dma_start(out=cw, in_=convw[:, :])
        bf = const.tile([P, CT], fp32, tag="bf")
        nc.sync.dma_start(out=bf, in_=biasf[:, :])
        hl = const.tile([P, CT * 3], fp32, tag="hl")
        nc.sync.dma_start(out=hl, in_=halo[:, :])
        bo = const.tile([P, MT], fp32, tag="bo")
        nc.sync.dma_start(out=bo, in_=bout[:, :])
        # last 3 conv-input columns of each channel tile, carried across supers
        xtail = const.tile([P, CT * 3], fp32, tag="xtail")

        for s in range(NSUP):
            hst = hs_pool.tile([P, KH * SUP], bf16, tag="hs")

            def hA(k):
                return hst[:, k * SUP:k * SUP + SUB]

            def hB(k):
                return hst[:, k * SUP + SUB:(k + 1) * SUP]

            # DMA batch order matched to the product schedule's needs:
            # b1=hA(0-7) [M2], b2=hB(8-15) [M5], b3=hA(8-15) [T4,T7],
            # b4=hB(0-7) [T3,T6].
            def _h_dma(k, hi, eng=None):
                (eng or nc.sync).dma_start(
                    out=hB(k) if hi else hA(k),
                    in_=hsT[
                        k * P:(k + 1) * P,
                        s * SUP + (SUB if hi else 0):s * SUP
                        + (SUP if hi else SUB),
                    ],
                )

            # Super-0 prologue: spread the load over both HW-DGE rings and
            # prefetch r=0's stationaries around the hst batches so every
            # r=0 product finds operands as its PE slot arrives.
            pre_st = {}

            def _pre_st(ps, eng):
                for p in ps:
                    st0 = sp_pool.tile([P, KQ * P], bf16, tag="s1", name="stp")
                    eng.dma_start(out=st0, in_=w1s[p * RT][:, :])
                    pre_st[p] = st0

            if s == 0:
                # p3/p4 (M4/M7) must beat b2 on the ring or they become the
                # critical path; p2 (M1) is gated on the full b2 anyway.
                _pre_st(PRE_ACT_EARLY, nc.scalar)
            for k in range(KQ):
                _h_dma(k, hi=False)          # b1: hA(0-7) on SP

            # Moving-side Strassen combos (bf16, built once per super on DVE).
            combos = {
                nm: tc_pool.tile([P, KQ * SUB], bf16, tag=nm, name=nm)
                for nm in ("T1", "T6", "T3", "T4", "T7")
            }

            def cslice(nm, k):
                return combos[nm][:, k * SUB:(k + 1) * SUB]

            # Remaining hst batches split across the two rings, with each
            # combo k-slice built right after the DMA completing it, so
            # combos track DMA arrival instead of trailing the super load.
            for k in range(KQ, KH):
                _h_dma(k, hi=False)          # b3: hA(8-15) on SP
            if s == 0:
                _pre_st((5, 6), nc.sync)
            for k in range(KQ):
                nc.vector.tensor_sub(cslice("T4", k), hA(KQ + k), hA(k))
            # T7/T6 build on the idle GpSimd so T3 (which gates product M3)
            # does not queue behind them on the in-order DVE; T1/T7 slices
            # build as each b2 DMA lands.
            for k in range(KQ, KH):
                _h_dma(k, hi=True, eng=nc.scalar)   # b2: hB(8-15) on Act
                nc.vector.tensor_add(cslice("T1", k - KQ), hA(k - KQ), hB(k))
                nc.gpsimd.tensor_tensor(
                    cslice("T7", k - KQ), hA(k), hB(k), op=ALU.add
                )
            if s == 0:
                _pre_st(PRE_ACT_LATE, nc.scalar)
            for k in range(KQ):
                _h_dma(k, hi=True, eng=nc.scalar)   # b4: hB(0-7) on Act
                nc.vector.tensor_sub(cslice("T3", k), hB(k), hB(KQ + k))
                nc.gpsimd.tensor_tensor(
                    cslice("T6", k), hA(k), hB(k), op=ALU.add
                )

            moving = {
                "M2": hA,
                "M5": lambda k: hB(KQ + k),
                "M6": lambda k: cslice("T6", k),
                "M3": lambda k: cslice("T3", k),
                "M4": lambda k: cslice("T4", k),
                "M1": lambda k: cslice("T1", k),
                "M7": lambda k: cslice("T7", k),
            }

            ybig = yb_pool.tile([P, CT * SUP], bf16, tag="yb")

            # Phase A: Strassen products + recombination into the two conv
            # input strips (ci=r from X11|X12, ci=16+r from X21|X22).
            for r in range(RT):
                xsA = xs_pool.tile([P, 3 + SUP], fp32, tag="xsA")
                xsB = xs_pool.tile([P, 3 + SUP], fp32, tag="xsB")
                x11 = xsA[:, 3:3 + SUB]
                x12 = xsA[:, 3 + SUB:3 + SUP]
                x21 = xsB[:, 3:3 + SUB]
                x22 = xsB[:, 3 + SUB:3 + SUP]
                for ci, xs in ((r, xsA), (RT + r, xsB)):
                    if s == 0:
                        nc.vector.tensor_copy(xs[:, 0:3], hl[:, ci * 3:ci * 3 + 3])
                    else:
                        nc.vector.tensor_copy(
                            xs[:, 0:3], xtail[:, ci * 3:ci * 3 + 3]
                        )

                def conv_silu_half(ci, xs, h):
                    # conv + SiLU on one 512-col half of a strip; emitted as
                    # soon as its xs region is complete so the ybig half-slice
                    # lands early for phase-B consumers. The tap work is
                    # spread over three engines to keep the DVE (which owns
                    # the Strassen recombines) off the critical path: strip B
                    # runs its mul/adds on the otherwise-idle GpSimd (plain
                    # tensor_tensor only — walrus rejects TensorScalarPtr on
                    # Pool, hence the broadcast_to weight APs), taps 1-2 are
                    # per-partition-scaled Identity activations on the
                    # scalar engine, and only the fused final tap is DVE.
                    strip_b = ci >= RT
                    off = h * SUB

                    def w(t):
                        return cw[:, ci * 4 + t:ci * 4 + t + 1]

                    ya = ya_pool.tile([P, SUB], fp32, tag="ya")
                    if strip_b:
                        nc.gpsimd.tensor_tensor(
                            ya, xs[:, off:off + SUB],
                            w(0).broadcast_to([P, SUB]), op=ALU.mult,
                        )
                        for t in (1, 2):
                            tm = ya_pool.tile([P, SUB], fp32, tag="tm")
                            nc.scalar.activation(
                                tm, xs[:, off + t:off + t + SUB], AF.Identity,
                                scale=w(t),
                            )
                            nc.gpsimd.tensor_tensor(ya, ya, tm, op=ALU.add)
                        nc.vector.scalar_tensor_tensor(
                            out=ya, in0=xs[:, off + 3:off + 3 + SUB],
                            scalar=w(3), in1=ya, op0=ALU.mult, op1=ALU.add,
                        )
                    else:
                        nc.vector.tensor_scalar_mul(
                            ya, xs[:, off:off + SUB], w(0)
                        )
                        for t in range(1, 4):
                            nc.vector.scalar_tensor_tensor(
                                out=ya, in0=xs[:, off + t:off + t + SUB],
                                scalar=w(t), in1=ya, op0=ALU.mult, op1=ALU.add,
                            )
                    nc.scalar.activation(
                        ybig[:, ci * SUP + off:ci * SUP + off + SUB],
                        ya,
                        AF.Silu,
                        bias=bf[:, ci:ci + 1],
                        scale=1.0,
                    )

                px = {}
                for p, nm in enumerate(PRODS):
                    if r == 0 and p in pre_st:
                        st = pre_st.pop(p)
                    else:
                        st = sp_pool.tile([P, KQ * P], bf16, tag="s1")
                        dma_eng = (
                            nc.scalar if (p % 2 == 0 or not ST_RING_SPLIT)
                            else nc.sync
                        )
                        dma_eng.dma_start(out=st, in_=w1s[p * RT + r][:, :])
                    pt = psA.tile([P, SUB], fp32, tag="px")
                    px[nm] = pt
                    mv = moving[nm]
                    for k in range(KQ):
                        nc.tensor.matmul(
                            pt,
                            st[:, k * P:(k + 1) * P],
                            mv(k),
                            start=(k == 0),
                            stop=(k == KQ - 1),
                        )
                    if nm == "M2":
                        nc.scalar.copy(x21, pt)
                    elif nm == "M5":
                        nc.scalar.copy(x12, pt)
                        nc.vector.tensor_scalar_mul(x11, pt, -1.0)
                    elif nm == "M1":
                        nc.scalar.copy(x22, pt)
                        nc.vector.scalar_tensor_tensor(
                            out=x22, in0=px["M2"], scalar=-1.0, in1=x22,
                            op0=ALU.mult, op1=ALU.add,
                        )
                        nc.vector.tensor_add(x11, x11, pt)
                    elif nm == "M4":
                        nc.vector.tensor_add(x11, x11, pt)
                        nc.vector.tensor_add(x21, x21, pt)
                        conv_silu_half(RT + r, xsB, 0)  # X21 complete
                    elif nm == "M7":
                        nc.vector.tensor_add(x11, x11, pt)
                        conv_silu_half(r, xsA, 0)       # X11 complete
                    elif nm == "M3":
                        nc.vector.tensor_add(x12, x12, pt)
                        nc.vector.tensor_add(x22, x22, pt)
                        conv_silu_half(r, xsA, 1)       # X12 complete
                    else:  # M6
                        nc.vector.tensor_add(x22, x22, pt)
                        conv_silu_half(RT + r, xsB, 1)  # X22 complete

                if s + 1 < NSUP:
                    for ci, xs in ((r, xsA), (RT + r, xsB)):
                        nc.vector.tensor_copy(
                            xtail[:, ci * 3:ci * 3 + 3], xs[:, SUP:SUP + 3]
                        )

            # Phase B: out = W_out @ Y (accumulate over all channel tiles)
            for m in range(MT):
                wo = wo_pool.tile([P, CT * P], bf16, tag="wo")
                for q in range(4):  # split across HW queues
                    nc.sync.dma_start(
                        out=wo[:, q * 8 * P:(q + 1) * 8 * P],
                        in_=w_out[m][:, q * 8 * P:(q + 1) * 8 * P],
                    )
                for sub in range(NSUB):
                    off = sub * SUB
                    po = psB.tile([P, SUB], fp32, tag="po")
                    # contract in ybig production order (ci=r, 16+r) so the
                    # scheduler can interleave these into phase A stall-free
                    for j, ci2 in enumerate(
                        c for r2 in range(RT) for c in (r2, RT + r2)
                    ):
                        nc.tensor.matmul(
                            po,
                            wo[:, ci2 * P:(ci2 + 1) * P],
                            ybig[:, ci2 * SUP + off:ci2 * SUP + off + SUB],
                            start=(j == 0),
                            stop=(j == CT - 1),
                        )
                    ob = ob_pool.tile([P, SUB], fp32, tag="ob")
                    nc.scalar.activation(
                        ob, po, AF.Identity, bias=bo[:, m:m + 1], scale=1.0
                    )
                    dst = outT[m * P:(m + 1) * P, s * SUP + off:s * SUP + off + SUB]
                    if s == NSUP - 1 and m == MT - 1:
                        # drain: split the tail DMAs across both HW rings
                        nc.sync.dma_start(out=dst[:, 0:SUB // 2], in_=ob[:, 0:SUB // 2])
                        nc.scalar.dma_start(out=dst[:, SUB // 2:], in_=ob[:, SUB // 2:])
                    else:
                        nc.sync.dma_start(out=dst, in_=ob)
    nc.finalize()
    return nc


def _prep_inputs(hidden_states, W_in, b_in, conv_w, conv_b, W_out, b_out):
    bf16 = ml_dtypes.bfloat16
    f32 = np.float32
    hidden_states = np.asarray(hidden_states, f32)
    W_in = np.asarray(W_in, f32)
    b_in = np.asarray(b_in, f32)
    conv_w = np.asarray(conv_w, f32)
    conv_b = np.asarray(conv_b, f32)
    W_out = np.asarray(W_out, f32)
    b_out = np.asarray(b_out, f32)

    # Strassen stationary combos, in the kernel's product schedule order.
    HC, HQ = C // 2, H // 2
    A11 = W_in[:HC, :HQ]
    A12 = W_in[:HC, HQ:]
    A21 = W_in[HC:, :HQ]
    A22 = W_in[HC:, HQ:]
    # stationary combos in PRODS order [M2, M5, M1, M4, M7, M3, M6]
    stat = [A21 + A22, A11 + A12, A11 + A22, A22, A12 - A22, A11, A21 - A11]
    w1s = np.stack(
        [
            np.ascontiguousarray(
                Si.reshape(RT, P, KQ, P).transpose(0, 3, 2, 1).reshape(RT, P, KQ * P)
            )
            for Si in stat
        ]
    ).reshape(7 * RT, P, KQ * P).astype(bf16)

    w_out2 = np.ascontiguousarray(
        W_out.reshape(MT, P, CT, P).transpose(0, 3, 2, 1).reshape(MT, P, CT * P)
    ).astype(bf16)
    wv = conv_w[:, 0, :]  # [C, 4]
    convw_all = np.ascontiguousarray(
        wv.reshape(CT, P, 4).transpose(1, 0, 2).reshape(P, CT * 4)
    ).astype(f32)
    biasf_all = np.ascontiguousarray(
        (conv_b + b_in * wv.sum(1)).reshape(CT, P).T
    ).astype(f32)
    bout2 = np.ascontiguousarray(b_out.reshape(MT, P).T).astype(f32)

    in_maps = []
    for core in range(NCORES):
        b, half = divmod(core, 2)
        hs = hidden_states[b, half * N:(half + 1) * N, :]
        hsT_arr = np.ascontiguousarray(hs.T).astype(bf16)
        if half == 0:
            xraw = np.repeat(-b_in[:, None], 3, axis=1)
        else:
            hs3 = hidden_states[b, half * N - 3:half * N, :]  # [3, H]
            xraw = W_in @ hs3.T  # [C, 3]
        halo_all = np.ascontiguousarray(
            xraw.reshape(CT, P, 3).transpose(1, 0, 2).reshape(P, CT * 3)
        ).astype(f32)
        in_maps.append(
            {
                "hsT": hsT_arr,
                "w1s": w1s,
                "w_out": w_out2,
                "convw": convw_all,
                "biasf": biasf_all,
                "halo": halo_all,
                "bout": bout2,
            }
        )
    return in_maps


def kernel(hidden_states, W_in, b_in, conv_w, conv_b, W_out, b_out, trace=False):
    global _NC, LAST_RESULT
    from concourse.bass_utils import run_bass_kernel_spmd

    if _NC is None:
        _NC = _build_nc()
    in_maps = _prep_inputs(
        hidden_states, W_in, b_in, conv_w, conv_b, W_out, b_out
    )
    res = run_bass_kernel_spmd(_NC, in_maps, list(range(NCORES)), trace=trace)
    LAST_RESULT = res
    out = np.empty((B, S, H), np.float32)
    for core in range(NCORES):
        b, half = divmod(core, 2)
        out[b, half * N:(half + 1) * N, :] = res.results[core]["outT"].T
    return out
